# revision 23
# baseline (speedup 1.0000x reference)
"""Trainium2 Bass kernel for nn_CandidateSubgraphTFModel (gnn_message_passing).

Strategy (8 NeuronCores, SPMD + one AllGather):
  - Data-parallel over batch B=32 -> 4 rows/core for the TCN/encode path.
  - GCN sharded over N=2048 candidate rows -> 256 rows/core. The host
    precomputes the degree vector d, gathers/scales X_tilde = d_j *
    embed[sub_nodes], transposes the A_hat shard, pre-casts weights and the
    embedding table to bf16, and lays large operands out so each loads with
    a single DMA.
  - hproj and LN gamma are folded into the candidate matrix
    G = g * (H @ hproj_w), all-gathered in bf16 while the encode convs run.
  - logits = (z - mu) @ GgT scaled by (rstd * mask) per token during the
    PSUM->SBUF copy, so LayerNorm costs no extra matmuls and no row-wise
    reciprocal sits on the DVE queue.
  - bf16 matmul datapath (fp32 accumulation in PSUM); logits stored bf16 and
    upcast on host. rel-err ~6e-3 against the 2e-2 gate.
  - PE-fullness: two-deep supertile software pipeline per iteration:
    transposes[t], logits[t-2], zc[t-1] (DVE), conv1[t], conv2[t], stats[t];
    the GCN is emitted right after conv1 of the first supertile.

kernel(**inputs) takes FULL inputs (as in reference.setup_inputs()) and
returns the FULL [32, 1024, 2048] float32 logits.
"""
import sys
sys.path.insert(0, '/opt/trn_rl_repo')
import numpy as np

import concourse.bass as bass
import concourse.bacc as bacc
import concourse.tile as tile
from concourse import mybir
from concourse.masks import make_identity

f32 = mybir.dt.float32
f32r = mybir.dt.float32r
bf16 = mybir.dt.bfloat16
i32 = mybir.dt.int32
AF = mybir.ActivationFunctionType
OP = mybir.AluOpType

LN_EPS = 1e-5


class Cfg:
    def __init__(self, V=50000, D=512, B=32, S=1024, N=2048, K=3, n_cores=8):
        self.V, self.D, self.B, self.S, self.N, self.K = V, D, B, S, N, K
        self.n_cores = n_cores
        self.B_loc = B // n_cores
        self.DC = D // 128          # feature chunks
        self.ST = min(512, S)       # supertile (tokens)
        self.NTT = self.ST // 128   # token-tiles per supertile
        self.NST = S // self.ST     # supertiles per row
        self.NJC = N // 128         # GCN j chunks
        self.N_loc = N // n_cores   # GCN rows per core
        self.SW = min(512, N)       # logits slice width over N
        self.NSL = N // self.SW
        self.TOK = self.B_loc * S   # tokens per core


def build_program(cfg, reps=1, has_c0=False):
    c = cfg
    nc = bacc.Bacc("TRN2", target_bir_lowering=False, debug=False,
                   num_devices=c.n_cores)

    aps = {
        'x_in': nc.dram_tensor("x_in_loc", [c.TOK], i32,
                               kind="ExternalInput").ap(),
        'mask': nc.dram_tensor("mask_loc", [c.TOK], f32,
                               kind="ExternalInput").ap(),
        # [128, NJC, N_loc]: A_hatT shard, partition-major relayout
        'A_locT': nc.dram_tensor("A_locTp", [128, c.NJC, c.N_loc], bf16,
                                 kind="ExternalInput").ap(),
        # [128, NJC, D]: X_tilde, partition-major relayout
        'X_t': nc.dram_tensor("X_tp", [128, c.NJC, c.D], bf16,
                              kind="ExternalInput").ap(),
        'd_loc': nc.dram_tensor("d_loc", [1, c.N_loc], f32,
                                kind="ExternalInput").ap(),
        'embed': nc.dram_tensor("embed_bf", [c.V, c.D], bf16,
                                kind="ExternalInput").ap(),
        # [128, K, DC, D] partition-major conv weights
        'c1w': nc.dram_tensor("conv1_wp", [128, c.K, c.DC, c.D], bf16,
                              kind="ExternalInput").ap(),
        'c1b': nc.dram_tensor("conv1_b", [c.D], f32,
                              kind="ExternalInput").ap(),
        'c2w': nc.dram_tensor("conv2_wp", [128, c.K, c.DC, c.D], bf16,
                              kind="ExternalInput").ap(),
        'c2b': nc.dram_tensor("conv2_b", [c.D], f32,
                              kind="ExternalInput").ap(),
        'ln_g': nc.dram_tensor("ln_g", [c.D], f32, kind="ExternalInput").ap(),
        'ln_b': nc.dram_tensor("ln_b", [c.D], f32, kind="ExternalInput").ap(),
        # [128, DC, D] partition-major gcn_w^T / hproj_w
        'gwT': nc.dram_tensor("gcn_wTp", [128, c.DC, c.D], bf16,
                              kind="ExternalInput").ap(),
        'gb': nc.dram_tensor("gcn_b", [c.D], f32, kind="ExternalInput").ap(),
        'hw_w': nc.dram_tensor("hproj_wp", [128, c.DC, c.D], bf16,
                               kind="ExternalInput").ap(),
        'out': nc.dram_tensor("logits_loc", [c.TOK, c.N], bf16,
                              kind="ExternalOutput").ap(),
    }

    with tile.TileContext(nc) as tc:
        for _ in range(reps):
            build_body(tc, c, aps, has_c0)
    nc.compile()
    return nc


def build_body(tc, c, aps, has_c0=False):
    nc = tc.nc
    DC, K, ST, NTT, NST, NJC, SW, NSL = (c.DC, c.K, c.ST, c.NTT, c.NST,
                                         c.NJC, c.SW, c.NSL)
    NL = c.N_loc
    x_in, mask = aps['x_in'], aps['mask']
    A_locT, X_t, d_loc, embed = (aps['A_locT'], aps['X_t'], aps['d_loc'],
                                 aps['embed'])
    c1w, c1b, c2w, c2b = aps['c1w'], aps['c1b'], aps['c2w'], aps['c2b']
    ln_g, ln_b, gwT, gb, hw_w = (aps['ln_g'], aps['ln_b'], aps['gwT'],
                                 aps['gb'], aps['hw_w'])
    out = aps['out']

    # ---------------- pools ----------------
    const = tc.alloc_tile_pool(name="const", bufs=1)
    ggp = tc.alloc_tile_pool(name="ggp", bufs=1)
    wp = tc.alloc_tile_pool(name="wp", bufs=1)
    gp = tc.alloc_tile_pool(name="gp", bufs=1)
    dram = tc.alloc_tile_pool(name="dram", bufs=1, space="DRAM")
    ep = tc.alloc_tile_pool(name="ep", bufs=1)
    pe = tc.alloc_tile_pool(name="pe", bufs=1, space="PSUM")

    # ---------------- constants ----------------
    ident = const.tile([128, 128], f32)
    make_identity(nc, ident[:])
    identb = const.tile([128, 128], bf16)
    nc.vector.tensor_copy(out=identb[:], in_=ident[:])
    ones_f = const.tile([128, 1], f32)
    nc.vector.memset(ones_f[:], 1.0)
    onesr_f = const.tile([1, 128], f32)
    nc.vector.memset(onesr_f[:], 1.0)
    ones_col = const.tile([128, 1], bf16)
    nc.vector.tensor_copy(out=ones_col[:], in_=ones_f[:])
    ones_row = const.tile([1, 128], f32r)
    nc.vector.tensor_copy(out=ones_row[:], in_=onesr_f[:])
    zpad = const.tile([128, 2], bf16)
    nc.vector.memset(zpad[:], 0.0)

    # Warm the activation tables (Relu/Square/Sqrt) off the critical path.
    zc1 = const.tile([1, 1], f32)
    nc.vector.memset(zc1[:], 0.0)
    warm = const.tile([1, 8], f32)
    nc.vector.memset(warm[:], 1.0)
    warm2 = const.tile([1, 8], f32)
    nc.scalar.square(warm2[:], warm[:])
    nc.scalar.sqrt(warm2[:], warm[:])
    nc.scalar.activation(out=warm2[:], in_=warm[:], func=AF.Relu,
                         bias=zc1[:])

    def load_cols(dram_vec, name):
        cols = []
        for dc in range(DC):
            t = const.tile([128, 1], f32, name=f"{name}_{dc}")
            nc.sync.dma_start(out=t[:], in_=dram_vec[dc*128:(dc+1)*128, None])
            cols.append(t)
        return cols

    b1_col = load_cols(c1b, "b1")
    b2_col = load_cols(c2b, "b2")
    g_col = load_cols(ln_g, "g")
    gb_col = load_cols(gb, "gb")
    lnb_col = load_cols(ln_b, "lnb") if has_c0 else None

    # GgT[dc] : [128, n_cores, N_loc] bf16 (candidate matrix, feature-major)
    GgT = [ggp.tile([128, c.n_cores, NL], bf16, name=f"GgT_{dc}")
           for dc in range(DC)]

    # =================== per-supertile gather (hoistable) ===============
    def emit_gather(row, st, x_fm):
        row0, s0 = row * c.S, st * ST
        mcols = []
        for tt in range(NTT):
            t0 = s0 + tt * 128
            idx = ep.tile([128, 1], i32, name="idx", bufs=4)
            nc.scalar.dma_start(out=idx[:],
                                in_=x_in[row0+t0:row0+t0+128, None])
            mc = ep.tile([128, 1], f32, name="mc", bufs=12)
            nc.scalar.dma_start(out=mc[:],
                                in_=mask[row0+t0:row0+t0+128, None])
            mcols.append(mc)
            x_tm = ep.tile([128, c.D], bf16, name="x_tm", bufs=4)
            nc.gpsimd.indirect_dma_start(
                out=x_tm[:], out_offset=None, in_=embed[:],
                in_offset=bass.IndirectOffsetOnAxis(ap=idx[:, :1], axis=0))
            xm = ep.tile([128, c.D], bf16, name="xm", bufs=3)
            nc.vector.tensor_scalar_mul(xm[:], x_tm[:], mc[:])
            tp_ps = pe.tile([128, c.D], bf16, name="tp_ps", tag="c2s", bufs=2)
            for dc in range(DC):
                nc.tensor.transpose(out=tp_ps[:, dc*128:(dc+1)*128],
                                    in_=xm[:, dc*128:(dc+1)*128],
                                    identity=identb[:])
            for dc in range(DC):
                nc.vector.tensor_copy(
                    out=x_fm[dc][:, 2+t0:2+t0+128],
                    in_=tp_ps[:, dc*128:(dc+1)*128])
        return mcols

    def alloc_rowbuf():
        x_fm = [ep.tile([128, c.S + 2], bf16, name=f"xfm_{dcc}", bufs=1)
                for dcc in range(DC)]
        y1_fm = [ep.tile([128, c.S + 2], bf16, name=f"y1fm_{dcc}", bufs=1)
                 for dcc in range(DC)]
        for dc in range(DC):
            nc.vector.tensor_copy(out=x_fm[dc][:, 0:2], in_=zpad[:, :])
            nc.vector.tensor_copy(out=y1_fm[dc][:, 0:2], in_=zpad[:, :])
        return x_fm, y1_fm

    # Hoist row-0 buffers + the first supertile's gather ahead of the
    # weight loads so the PE can start transposing at once.
    rowbuf = {0: alloc_rowbuf()}
    pre_mcols = emit_gather(0, 0, rowbuf[0][0])

    # weight loads split across both HWDGE queues (conv1 gates iter 0)
    w1t = wp.tile([128, K, DC, c.D], bf16, name="w1t")
    nc.sync.dma_start(out=w1t[:, 0:2, :, :], in_=c1w[:, 0:2, :, :])
    nc.scalar.dma_start(out=w1t[:, 2:3, :, :], in_=c1w[:, 2:3, :, :])
    w2t = wp.tile([128, K, DC, c.D], bf16, name="w2t")
    nc.scalar.dma_start(out=w2t[:, 0:2, :, :], in_=c2w[:, 0:2, :, :])
    nc.sync.dma_start(out=w2t[:, 2:3, :, :], in_=c2w[:, 2:3, :, :])
    gwt = gp.tile([128, DC, c.D], bf16, name="gwt")
    nc.sync.dma_start(out=gwt[:], in_=gwT[:])
    hwt = gp.tile([128, DC, c.D], bf16, name="hwt")
    nc.scalar.dma_start(out=hwt[:], in_=hw_w[:])

    # =================== GCN (emitted inside first supertile) ===========
    def emit_gcn():
        # X_tilde and A_hatT shard arrive pre-scaled / pre-transposed bf16.
        xtt = gp.tile([128, NJC, c.D], bf16, name="xtt")
        nc.sync.dma_start(out=xtt[:, 0:NJC//2, :], in_=X_t[:, 0:NJC//2, :])
        nc.scalar.dma_start(out=xtt[:, NJC//2:, :], in_=X_t[:, NJC//2:, :])
        att = gp.tile([128, NJC, NL], bf16, name="att")
        nc.scalar.dma_start(out=att[:, 0:NJC//2, :], in_=A_locT[:, 0:NJC//2, :])
        nc.sync.dma_start(out=att[:, NJC//2:, :], in_=A_locT[:, NJC//2:, :])

        # M1T[d, i] = sum_j Xt[j, d] * A_hatT[j, i]
        m1 = []
        for dc in range(DC):
            m1_ps = pe.tile([128, NL], f32, name="m1p", tag="c1", bufs=2)
            for jc in range(NJC):
                nc.tensor.matmul(out=m1_ps[:],
                                 lhsT=xtt[:, jc, dc*128:(dc+1)*128],
                                 rhs=att[:, jc, :], start=(jc == 0),
                                 stop=(jc == NJC - 1))
            m1s = gp.tile([128, NL], bf16, name=f"m1s_{dc}")
            nc.scalar.copy(out=m1s[:], in_=m1_ps[:])
            m1.append(m1s)

        # d_i broadcast [128, N_loc]
        dl_f = gp.tile([1, NL], f32, name="dl_f")
        nc.scalar.dma_start(out=dl_f[:], in_=d_loc[0:1, :])
        dl_r = gp.tile([1, NL], f32r, name="dl_r")
        nc.vector.tensor_copy(out=dl_r[:], in_=dl_f[:])
        db_ps = pe.tile([128, NL], f32, name="db_ps", tag="lg", bufs=2)
        nc.tensor.matmul(out=db_ps[:], lhsT=ones_row[:], rhs=dl_r[:],
                         start=True, stop=True)
        db = gp.tile([128, NL], f32, name="db")
        nc.vector.tensor_copy(out=db[:], in_=db_ps[:])

        # H_shard = relu(d_i * (gw @ M1T) + gcn_b)   [e, i] bf16
        HT = []
        for ec in range(DC):
            h2_ps = pe.tile([128, NL], f32, name=f"h2p_{ec}", tag="lg",
                            bufs=2)
            for dc in range(DC):
                nc.tensor.matmul(out=h2_ps[:],
                                 lhsT=gwt[:, dc, ec*128:(ec+1)*128],
                                 rhs=m1[dc][:], start=(dc == 0),
                                 stop=(dc == DC - 1))
            hd = gp.tile([128, NL], f32, name="hd", bufs=2)
            nc.vector.tensor_mul(hd[:], h2_ps[:], db[:])
            ht = gp.tile([128, NL], bf16, name=f"ht_{ec}")
            nc.scalar.activation(out=ht[:], in_=hd[:], func=AF.Relu,
                                 bias=gb_col[ec][:])
            HT.append(ht)

        # G_shard^T[d, i] = sum_e W[e, d] * H_shard[e, i]; scale by gamma[d]
        gg_in = dram.tile([c.D, NL], bf16)
        gg_all = dram.tile([c.n_cores, c.D, NL], bf16, addr_space="Shared")
        for dc in range(DC):
            g2_ps = pe.tile([128, NL], f32, name=f"g2p_{dc}", tag="lg",
                            bufs=2)
            for ec in range(DC):
                nc.tensor.matmul(out=g2_ps[:],
                                 lhsT=hwt[:, ec, dc*128:(dc+1)*128],
                                 rhs=HT[ec][:], start=(ec == 0),
                                 stop=(ec == DC - 1))
            gg_sb = gp.tile([128, NL], bf16, name="gg_sb", bufs=2)
            if has_c0:
                nc.vector.tensor_copy(out=gg_sb[:], in_=g2_ps[:])
            else:
                nc.vector.tensor_scalar_mul(gg_sb[:], g2_ps[:], g_col[dc][:])
            nc.gpsimd.dma_start(out=gg_in[dc*128:(dc+1)*128, :], in_=gg_sb[:])

        nc.gpsimd.collective_compute(
            "AllGather", mybir.AluOpType.bypass,
            replica_groups=[list(range(c.n_cores))],
            ins=[gg_in[:].opt()],
            outs=[gg_all[:].opt()])

        # read back c2-major so the first logits slice is ready earliest
        for c2 in range(c.n_cores):
            for dc in range(DC):
                eng = nc.scalar if (c2 * DC + dc) % 2 == 0 else nc.sync
                eng.dma_start(out=GgT[dc][:, c2, :],
                              in_=gg_all[c2, dc*128:(dc+1)*128, :])

    # c0 row broadcast tiles (only when ln_b != 0)
    c0_bc = []

    def emit_c0():
        lnb_r = []
        for dc in range(DC):
            lr = ep.tile([128, 1], bf16, name=f"lnb_r_{dc}")
            nc.vector.tensor_copy(out=lr[:], in_=lnb_col[dc][:])
            lnb_r.append(lr)
        for ns in range(NSL):
            c0_ps = pe.tile([1, SW], f32, name="c0_ps", tag="sbc", bufs=2)
            for dc in range(DC):
                nc.tensor.matmul(out=c0_ps[0:1, :], lhsT=lnb_r[dc][:],
                                 rhs=GgT[dc][:, 2*ns:2*ns+2, :],
                                 start=(dc == 0), stop=(dc == DC - 1))
            c0_row = ep.tile([1, SW], f32r, name="c0_row", bufs=2)
            nc.vector.tensor_copy(out=c0_row[:], in_=c0_ps[:])
            cb_ps = pe.tile([128, SW], f32, name="cb_ps", tag="sbc", bufs=2)
            nc.tensor.matmul(out=cb_ps[:], lhsT=ones_row[:],
                             rhs=c0_row[0:1, :], start=True, stop=True)
            cb = ep.tile([128, SW], f32, name=f"c0bc_{ns}")
            nc.vector.tensor_copy(out=cb[:], in_=cb_ps[:])
            c0_bc.append(cb)

    # =================== encode + logits =================
    iters = [(row, st) for row in range(c.B_loc) for st in range(NST)]
    zc_pend = [None]
    lg_pend = [None, None]

    for i, (row, st) in enumerate(iters):
        if st == 0 and row not in rowbuf:
            rowbuf[row] = alloc_rowbuf()
        x_fm, y1_fm = rowbuf[row]
        row0, s0 = row * c.S, st * ST

        # ---- gather + transposes (hoisted for the very first supertile) --
        if i == 0:
            mcols = pre_mcols
        else:
            mcols = emit_gather(row, st, x_fm)

        # ---- zc of ST t-1 (DVE), then logits of ST t-2 (PE) ----
        if zc_pend[0] is not None:
            zc_pend[0]()
            zc_pend[0] = None
        if lg_pend[0] is not None:
            lg_pend[0]()
        lg_pend[0] = lg_pend[1]
        lg_pend[1] = None

        # ---- conv1 (fm out) + relu ----
        for dc in range(DC):
            c1_ps = pe.tile([128, ST], f32, name="c1_ps", tag="c1", bufs=2)
            first = True
            for k in range(K):
                for dci in range(DC):
                    nc.tensor.matmul(
                        out=c1_ps[:],
                        lhsT=w1t[:, k, dci, dc*128:(dc+1)*128],
                        rhs=x_fm[dci][:, s0+k:s0+k+ST],
                        start=first, stop=(k == K-1 and dci == DC-1))
                    first = False
            nc.scalar.activation(out=y1_fm[dc][:, 2+s0:2+s0+ST],
                                 in_=c1_ps[:], func=AF.Relu,
                                 bias=b1_col[dc][:])

        if i == 0:
            emit_gcn()

        # ---- conv2 (fm out) + bias + residual -> z ----
        z = []
        for dc in range(DC):
            c2_ps = pe.tile([128, ST], f32, name="c2_ps", tag="c2s", bufs=2)
            first = True
            for k in range(K):
                for dci in range(DC):
                    nc.tensor.matmul(
                        out=c2_ps[:],
                        lhsT=w2t[:, k, dci, dc*128:(dc+1)*128],
                        rhs=y1_fm[dci][:, s0+k:s0+k+ST],
                        start=first, stop=(k == K-1 and dci == DC-1))
                    first = False
            zt = ep.tile([128, ST], bf16, name=f"z_{dc}", bufs=2)
            nc.vector.scalar_tensor_tensor(
                out=zt[:], in0=c2_ps[:], scalar=b2_col[dc][:],
                in1=x_fm[dc][:, 2+s0:2+s0+ST],
                op0=OP.add, op1=OP.add)
            z.append(zt)

        # ---- LN stats via PE matvecs (sum, then sumsq) ----
        st_ps = pe.tile([1, ST], f32, name="st_ps", tag="sbc", bufs=2)
        for dc in range(DC):
            nc.tensor.matmul(out=st_ps[0:1, :], lhsT=ones_col[:],
                             rhs=z[dc][:], start=(dc == 0),
                             stop=(dc == DC-1))
        zsq = []
        for dc in range(DC):
            zq = ep.tile([128, ST], bf16, name="zsq", bufs=2)
            nc.scalar.square(zq[:], z[dc][:])
            zsq.append(zq)
        sq_ps = pe.tile([1, ST], f32, name="sq_ps", tag="sbc", bufs=2)
        for dc in range(DC):
            nc.tensor.matmul(out=sq_ps[0:1, :], lhsT=ones_col[:],
                             rhs=zsq[dc][:], start=(dc == 0),
                             stop=(dc == DC-1))

        # ---- LN: mu, var; per-token-tile rstd*mask columns ----
        mu = ep.tile([1, ST], f32r, name="mu", bufs=2)
        nc.scalar.mul(mu[:], st_ps[0:1, :], 1.0 / c.D)
        ms = ep.tile([1, ST], f32, name="ms", bufs=2)
        nc.scalar.mul(ms[:], sq_ps[0:1, :], 1.0 / c.D)
        musq = ep.tile([1, ST], f32, name="musq", bufs=2)
        nc.scalar.square(musq[:], mu[:].bitcast(f32))
        nc.vector.scalar_tensor_tensor(out=ms[:], in0=ms[:],
                                       scalar=LN_EPS, in1=musq[:],
                                       op0=OP.add, op1=OP.subtract)
        rsm = []
        for tt in range(NTT):
            v_ps = pe.tile([128, 1], f32, name="v_ps", tag="sbc", bufs=2)
            nc.tensor.transpose(out=v_ps[:],
                                in_=ms[0:1, tt*128:(tt+1)*128],
                                identity=ident[0:1, 0:1])
            vr = ep.tile([128, 1], f32, name="vr", bufs=4)
            nc.vector.reciprocal(vr[:], v_ps[:])
            rs = ep.tile([128, 1], f32, name="rs", bufs=4)
            nc.scalar.sqrt(rs[:], vr[:])
            rm = ep.tile([128, 1], f32, name="rm", bufs=12)
            nc.vector.tensor_mul(rm[:], rs[:], mcols[tt][:])
            rsm.append(rm)

        # mu broadcast for the zc subtraction (consumed next iteration)
        mu_bc = pe.tile([128, ST], f32, name="mu_bc", tag="sbc", bufs=2)
        nc.tensor.matmul(out=mu_bc[:], lhsT=ones_row[:], rhs=mu[:],
                         start=True, stop=True)

        zc = [ep.tile([128, ST], bf16, name=f"zc_{dc}", bufs=3)
              for dc in range(DC)]

        def emit_zc(z=z, zc=zc, mu_bc=mu_bc):
            for dc in range(DC):
                if has_c0:
                    t1 = ep.tile([128, ST], f32, name="zc_t1", bufs=2)
                    nc.vector.tensor_tensor(out=t1[:], in0=z[dc][:],
                                            in1=mu_bc[:], op=OP.subtract)
                    nc.vector.tensor_scalar_mul(zc[dc][:], t1[:],
                                                g_col[dc][:])
                else:
                    nc.vector.tensor_tensor(out=zc[dc][:], in0=z[dc][:],
                                            in1=mu_bc[:], op=OP.subtract)

        def emit_logits(zc=zc, rsm=rsm, row0=row0, s0=s0, mcols=mcols):
            for tt in range(NTT):
                lo = ep.tile([128, c.N], bf16, name="lo", bufs=3)
                for ns in range(NSL):
                    lg_ps = pe.tile([128, SW], f32, name="lg_ps", tag="lg",
                                    bufs=2)
                    for dc in range(DC):
                        nc.tensor.matmul(
                            out=lg_ps[:],
                            lhsT=zc[dc][:, tt*128:(tt+1)*128],
                            rhs=GgT[dc][:, 2*ns:2*ns+2, :],
                            start=(dc == 0), stop=(dc == DC-1))
                    if has_c0:
                        t2 = ep.tile([128, SW], f32, name="lg_t2", bufs=2)
                        nc.vector.tensor_scalar_mul(t2[:], lg_ps[:],
                                                    rsm[tt][:])
                        nc.vector.scalar_tensor_tensor(
                            out=lo[:, ns*SW:(ns+1)*SW], in0=c0_bc[ns][:],
                            scalar=mcols[tt][:], in1=t2[:],
                            op0=OP.mult, op1=OP.add)
                    elif ns % 2 == 0:
                        nc.scalar.mul(lo[:, ns*SW:(ns+1)*SW], lg_ps[:],
                                      rsm[tt][:])
                    else:
                        nc.vector.tensor_scalar_mul(lo[:, ns*SW:(ns+1)*SW],
                                                    lg_ps[:], rsm[tt][:])
                t0g = row0 + s0 + tt * 128
                nc.sync.dma_start(out=out[t0g:t0g+128, :], in_=lo[:])

        zc_pend[0] = emit_zc
        lg_pend[1] = emit_logits

        if i == 0 and has_c0:
            emit_c0()

    if zc_pend[0] is not None:
        zc_pend[0]()
    if lg_pend[0] is not None:
        lg_pend[0]()
    if lg_pend[1] is not None:
        lg_pend[1]()

    pe.release()
    ep.release()
    dram.release()
    gp.release()
    wp.release()
    ggp.release()
    const.release()


# ---------------------------------------------------------------------------
# host side
# ---------------------------------------------------------------------------

_CACHE = {}
_BF16 = mybir.dt.np(mybir.dt.bfloat16)


def _get_program(cfg, has_c0=False):
    key = (cfg.V, cfg.D, cfg.B, cfg.S, cfg.N, cfg.K, cfg.n_cores, has_c0)
    if key not in _CACHE:
        _CACHE[key] = build_program(cfg, has_c0=has_c0)
    return _CACHE[key]


class _Runner:
    """Direct PJRT execution (no donation) so repeated runs are cheap."""

    def __init__(self, nc, n_cores):
        import jax
        from jax.sharding import Mesh, PartitionSpec, NamedSharding
        from jax.experimental.shard_map import shard_map
        from concourse import bass2jax
        bass2jax.install_neuronx_cc_hook()
        self.jax = jax
        self.n_cores = n_cores
        part_name = nc.partition_id_tensor.name if nc.partition_id_tensor else None
        in_names, out_names, out_avals, zero_outs = [], [], [], []
        for alloc in nc.m.functions[0].allocations:
            if not isinstance(alloc, mybir.MemoryLocationSet):
                continue
            name = alloc.memorylocations[0].name
            if alloc.kind == "ExternalInput":
                if name != part_name:
                    in_names.append(name)
            elif alloc.kind == "ExternalOutput":
                out_names.append(name)
                shape = tuple(alloc.tensor_shape)
                dtype = mybir.dt.np(alloc.dtype)
                out_avals.append(jax.core.ShapedArray(shape, dtype))
                zero_outs.append(np.zeros(shape, dtype))
        self.in_names, self.out_names = in_names, out_names
        self.out_avals, self.zero_outs = out_avals, zero_outs
        self.n_params = len(in_names)
        all_in = list(in_names) + list(out_names)
        if part_name:
            all_in.append(part_name)
        out_avals_t, all_in_t, out_names_t = (tuple(out_avals), tuple(all_in),
                                              tuple(out_names))

        def _body(*args):
            operands = list(args)
            if part_name:
                operands.append(bass2jax.partition_id_tensor())
            return tuple(bass2jax._bass_exec_p.bind(
                *operands, out_avals=out_avals_t, in_names=all_in_t,
                out_names=out_names_t, lowering_input_output_aliases=(),
                sim_require_finite=True, sim_require_nnan=True, nc=nc))

        devices = jax.devices()[:n_cores]
        self.mesh = Mesh(np.asarray(devices), ("core",))
        n_io = self.n_params + len(out_names)
        self.sharded = jax.jit(
            shard_map(_body, mesh=self.mesh,
                      in_specs=(PartitionSpec("core"),) * n_io,
                      out_specs=(PartitionSpec("core"),) * len(out_names),
                      check_rep=False),
            keep_unused=True)
        self.shard = NamedSharding(self.mesh, PartitionSpec("core"))

    def set_inputs(self, in_maps):
        jax = self.jax
        per_core = [[np.asarray(m[n]) for n in self.in_names] for m in in_maps]
        concat = [np.concatenate([per_core[cc][i] for cc in range(self.n_cores)],
                                 axis=0) for i in range(self.n_params)]
        concat += [np.zeros((self.n_cores * z.shape[0], *z.shape[1:]), z.dtype)
                   for z in self.zero_outs]
        self.dev_in = [jax.device_put(a, self.shard) for a in concat]
        jax.block_until_ready(self.dev_in)

    def run(self):
        outs = self.sharded(*self.dev_in)
        self.jax.block_until_ready(outs)
        return outs

    def run_np(self):
        outs = self.run()
        return [
            {n: np.asarray(outs[i]).reshape(self.n_cores,
                                            *self.out_avals[i].shape)[cc]
             for i, n in enumerate(self.out_names)}
            for cc in range(self.n_cores)
        ]


_RUNNER = {}


def _pmajor(arr, npart=128):
    """[n*128, rest...] -> [128, n, rest...] partition-major relayout."""
    n = arr.shape[0] // npart
    return np.ascontiguousarray(
        arr.reshape(n, npart, *arr.shape[1:]).swapaxes(0, 1))


def make_in_maps(cfg, inputs):
    c = cfg
    x_in = np.asarray(inputs['x_in'])
    mask = np.asarray(inputs['mask_in']).astype(np.float32)
    A = np.asarray(inputs['A_sub']).astype(np.float32)
    embed = np.asarray(inputs['embed']).astype(np.float32)
    sub_nodes = np.asarray(inputs['sub_nodes']).astype(np.int64)
    d_full = 1.0 / np.sqrt(np.maximum(A.sum(axis=1) + 1.0, 1e-6))
    d_full = d_full.astype(np.float32)
    X_t = (embed[sub_nodes] * d_full[:, None]).astype(_BF16)
    gcn_wT = np.ascontiguousarray(np.asarray(inputs['gcn_w']).T)
    w1 = np.asarray(inputs['conv1_w']).astype(_BF16)   # [K, D, D]
    w2 = np.asarray(inputs['conv2_w']).astype(_BF16)
    # [K, D, D] -> [128, K, DC, D]
    def conv_pm(w):
        return np.ascontiguousarray(
            w.reshape(c.K, c.DC, 128, c.D).transpose(2, 0, 1, 3))
    shared = {
        'X_tp': _pmajor(X_t),
        'embed_bf': embed.astype(_BF16),
        'conv1_wp': conv_pm(w1),
        'conv1_b': np.asarray(inputs['conv1_b']).astype(np.float32),
        'conv2_wp': conv_pm(w2),
        'conv2_b': np.asarray(inputs['conv2_b']).astype(np.float32),
        'ln_g': np.asarray(inputs['ln_g']).astype(np.float32),
        'ln_b': np.asarray(inputs['ln_b']).astype(np.float32),
        'gcn_wTp': _pmajor(gcn_wT.astype(_BF16)),
        'gcn_b': np.asarray(inputs['gcn_b']).astype(np.float32),
        'hproj_wp': _pmajor(np.asarray(inputs['hproj_w']).astype(_BF16)),
    }
    in_maps = []
    for cc in range(c.n_cores):
        rows = slice(cc * c.B_loc, (cc + 1) * c.B_loc)
        nrows = slice(cc * c.N_loc, (cc + 1) * c.N_loc)
        A_loc = A[nrows].copy()
        A_loc[np.arange(c.N_loc), np.arange(cc * c.N_loc,
                                            (cc + 1) * c.N_loc)] += 1.0
        m = dict(shared)
        m['x_in_loc'] = np.ascontiguousarray(
            x_in[rows].reshape(-1)).astype(np.int32)
        m['mask_loc'] = np.ascontiguousarray(mask[rows].reshape(-1))
        m['A_locTp'] = _pmajor(
            np.ascontiguousarray(A_loc.T).astype(_BF16))
        m['d_loc'] = d_full[nrows].reshape(1, -1).copy()
        in_maps.append(m)
    return in_maps


def kernel(**inputs):
    cfg = Cfg()
    has_c0 = bool(np.any(np.asarray(inputs['ln_b']) != 0))
    nc = _get_program(cfg, has_c0)
    key = id(nc)
    if key not in _RUNNER:
        _RUNNER[key] = _Runner(nc, cfg.n_cores)
    r = _RUNNER[key]
    r.set_inputs(make_in_maps(cfg, inputs))
    res = r.run_np()
    out = np.concatenate(
        [res[cc]['logits_loc'].astype(np.float32).reshape(cfg.B_loc, cfg.S,
                                                          cfg.N)
         for cc in range(cfg.n_cores)], axis=0)
    return out


# revision 24
# speedup vs baseline: 1.0178x; 1.0178x over previous
"""Trainium2 Bass kernel for nn_CandidateSubgraphTFModel (gnn_message_passing).

Strategy (8 NeuronCores, SPMD + one AllGather):
  - Data-parallel over batch B=32 -> 4 rows/core for the TCN/encode path.
  - GCN sharded over N=2048 candidate rows -> 256 rows/core. The host
    precomputes the degree vector d, gathers/scales X_tilde = d_j *
    embed[sub_nodes], transposes the A_hat shard, pre-casts weights and the
    embedding table to bf16, and lays large operands out so each loads with
    a single DMA.
  - hproj and LN gamma are folded into the candidate matrix
    G = g * (H @ hproj_w), all-gathered in bf16 while the encode convs run.
  - logits = (z - mu) @ GgT scaled by (rstd * mask) per token during the
    PSUM->SBUF copy, so LayerNorm costs no extra matmuls and no row-wise
    reciprocal sits on the DVE queue.
  - bf16 matmul datapath (fp32 accumulation in PSUM); logits stored bf16 and
    upcast on host. rel-err ~6e-3 against the 2e-2 gate.
  - PE-fullness: two-deep supertile software pipeline per iteration:
    transposes[t], logits[t-2], zc[t-1] (DVE), conv1[t], conv2[t], stats[t];
    the GCN is emitted right after conv1 of the first supertile.

kernel(**inputs) takes FULL inputs (as in reference.setup_inputs()) and
returns the FULL [32, 1024, 2048] float32 logits.
"""
import sys
sys.path.insert(0, '/opt/trn_rl_repo')
import numpy as np

import concourse.bass as bass
import concourse.bacc as bacc
import concourse.tile as tile
from concourse import mybir
from concourse.masks import make_identity

f32 = mybir.dt.float32
f32r = mybir.dt.float32r
bf16 = mybir.dt.bfloat16
i32 = mybir.dt.int32
AF = mybir.ActivationFunctionType
OP = mybir.AluOpType

LN_EPS = 1e-5


class Cfg:
    def __init__(self, V=50000, D=512, B=32, S=1024, N=2048, K=3, n_cores=8):
        self.V, self.D, self.B, self.S, self.N, self.K = V, D, B, S, N, K
        self.n_cores = n_cores
        self.B_loc = B // n_cores
        self.DC = D // 128          # feature chunks
        self.ST = min(512, S)       # supertile (tokens)
        self.NTT = self.ST // 128   # token-tiles per supertile
        self.NST = S // self.ST     # supertiles per row
        self.NJC = N // 128         # GCN j chunks
        self.N_loc = N // n_cores   # GCN rows per core
        self.SW = min(512, N)       # logits slice width over N
        self.NSL = N // self.SW
        self.TOK = self.B_loc * S   # tokens per core


def build_program(cfg, reps=1, has_c0=False):
    c = cfg
    nc = bacc.Bacc("TRN2", target_bir_lowering=False, debug=False,
                   num_devices=c.n_cores)

    aps = {
        'x_in': nc.dram_tensor("x_in_loc", [c.TOK], i32,
                               kind="ExternalInput").ap(),
        'mask': nc.dram_tensor("mask_loc", [c.TOK], f32,
                               kind="ExternalInput").ap(),
        # [128, NJC, N_loc]: A_hatT shard, partition-major relayout
        'A_locT': nc.dram_tensor("A_locTp", [128, c.NJC, c.N_loc], bf16,
                                 kind="ExternalInput").ap(),
        # [128, NJC, D]: X_tilde, partition-major relayout
        'X_t': nc.dram_tensor("X_tp", [128, c.NJC, c.D], bf16,
                              kind="ExternalInput").ap(),
        'd_loc': nc.dram_tensor("d_loc", [1, c.N_loc], f32,
                                kind="ExternalInput").ap(),
        'embed': nc.dram_tensor("embed_bf", [c.V, c.D], bf16,
                                kind="ExternalInput").ap(),
        # [128, K, DC, D] partition-major conv weights
        'c1w': nc.dram_tensor("conv1_wp", [128, c.K, c.DC, c.D], bf16,
                              kind="ExternalInput").ap(),
        'c1b': nc.dram_tensor("conv1_b", [c.D], f32,
                              kind="ExternalInput").ap(),
        'c2w': nc.dram_tensor("conv2_wp", [128, c.K, c.DC, c.D], bf16,
                              kind="ExternalInput").ap(),
        'c2b': nc.dram_tensor("conv2_b", [c.D], f32,
                              kind="ExternalInput").ap(),
        'ln_g': nc.dram_tensor("ln_g", [c.D], f32, kind="ExternalInput").ap(),
        'ln_b': nc.dram_tensor("ln_b", [c.D], f32, kind="ExternalInput").ap(),
        # [128, DC, D] partition-major gcn_w^T / hproj_w
        'gwT': nc.dram_tensor("gcn_wTp", [128, c.DC, c.D], bf16,
                              kind="ExternalInput").ap(),
        'gb': nc.dram_tensor("gcn_b", [c.D], f32, kind="ExternalInput").ap(),
        'hw_w': nc.dram_tensor("hproj_wp", [128, c.DC, c.D], bf16,
                               kind="ExternalInput").ap(),
        'out': nc.dram_tensor("logits_loc", [c.TOK, c.N], bf16,
                              kind="ExternalOutput").ap(),
    }

    with tile.TileContext(nc) as tc:
        for _ in range(reps):
            build_body(tc, c, aps, has_c0)
    nc.compile()
    return nc


def build_body(tc, c, aps, has_c0=False):
    nc = tc.nc
    DC, K, ST, NTT, NST, NJC, SW, NSL = (c.DC, c.K, c.ST, c.NTT, c.NST,
                                         c.NJC, c.SW, c.NSL)
    NL = c.N_loc
    x_in, mask = aps['x_in'], aps['mask']
    A_locT, X_t, d_loc, embed = (aps['A_locT'], aps['X_t'], aps['d_loc'],
                                 aps['embed'])
    c1w, c1b, c2w, c2b = aps['c1w'], aps['c1b'], aps['c2w'], aps['c2b']
    ln_g, ln_b, gwT, gb, hw_w = (aps['ln_g'], aps['ln_b'], aps['gwT'],
                                 aps['gb'], aps['hw_w'])
    out = aps['out']

    # ---------------- pools ----------------
    const = tc.alloc_tile_pool(name="const", bufs=1)
    ggp = tc.alloc_tile_pool(name="ggp", bufs=1)
    wp = tc.alloc_tile_pool(name="wp", bufs=1)
    gp = tc.alloc_tile_pool(name="gp", bufs=1)
    dram = tc.alloc_tile_pool(name="dram", bufs=1, space="DRAM")
    ep = tc.alloc_tile_pool(name="ep", bufs=1)
    pe = tc.alloc_tile_pool(name="pe", bufs=1, space="PSUM")

    # ---------------- constants ----------------
    ident = const.tile([128, 128], f32)
    make_identity(nc, ident[:])
    identb = const.tile([128, 128], bf16)
    nc.vector.tensor_copy(out=identb[:], in_=ident[:])
    ones_f = const.tile([128, 1], f32)
    nc.vector.memset(ones_f[:], 1.0)
    onesr_f = const.tile([1, 128], f32)
    nc.vector.memset(onesr_f[:], 1.0)
    ones_col = const.tile([128, 1], bf16)
    nc.vector.tensor_copy(out=ones_col[:], in_=ones_f[:])
    ones_row = const.tile([1, 128], f32r)
    nc.vector.tensor_copy(out=ones_row[:], in_=onesr_f[:])
    zpad = const.tile([128, 2], bf16)
    nc.vector.memset(zpad[:], 0.0)

    # Warm the activation tables (Relu/Square/Sqrt) off the critical path.
    zc1 = const.tile([1, 1], f32)
    nc.vector.memset(zc1[:], 0.0)
    warm = const.tile([1, 8], f32)
    nc.vector.memset(warm[:], 1.0)
    warm2 = const.tile([1, 8], f32)
    nc.scalar.square(warm2[:], warm[:])
    nc.scalar.sqrt(warm2[:], warm[:])
    nc.scalar.activation(out=warm2[:], in_=warm[:], func=AF.Relu,
                         bias=zc1[:])

    def load_cols(dram_vec, name):
        cols = []
        for dc in range(DC):
            t = const.tile([128, 1], f32, name=f"{name}_{dc}")
            nc.sync.dma_start(out=t[:], in_=dram_vec[dc*128:(dc+1)*128, None])
            cols.append(t)
        return cols

    b1_col = load_cols(c1b, "b1")
    b2_col = load_cols(c2b, "b2")
    g_col = load_cols(ln_g, "g")
    gb_col = load_cols(gb, "gb")
    lnb_col = load_cols(ln_b, "lnb") if has_c0 else None

    # GgT[dc] : [128, n_cores, N_loc] bf16 (candidate matrix, feature-major)
    GgT = [ggp.tile([128, c.n_cores, NL], bf16, name=f"GgT_{dc}")
           for dc in range(DC)]

    # =================== per-supertile gather (hoistable) ===============
    def emit_gather(row, st, x_fm):
        row0, s0 = row * c.S, st * ST
        mcols = []
        for tt in range(NTT):
            t0 = s0 + tt * 128
            idx = ep.tile([128, 1], i32, name="idx", bufs=4)
            nc.scalar.dma_start(out=idx[:],
                                in_=x_in[row0+t0:row0+t0+128, None])
            mc = ep.tile([128, 1], f32, name="mc", bufs=16)
            nc.scalar.dma_start(out=mc[:],
                                in_=mask[row0+t0:row0+t0+128, None])
            mcols.append(mc)
            x_tm = ep.tile([128, c.D], bf16, name="x_tm", bufs=4)
            nc.gpsimd.indirect_dma_start(
                out=x_tm[:], out_offset=None, in_=embed[:],
                in_offset=bass.IndirectOffsetOnAxis(ap=idx[:, :1], axis=0))
            xm = ep.tile([128, c.D], bf16, name="xm", bufs=3)
            nc.vector.tensor_scalar_mul(xm[:], x_tm[:], mc[:])
            tp_ps = pe.tile([128, c.D], bf16, name="tp_ps", tag="c2s", bufs=2)
            for dc in range(DC):
                nc.tensor.transpose(out=tp_ps[:, dc*128:(dc+1)*128],
                                    in_=xm[:, dc*128:(dc+1)*128],
                                    identity=identb[:])
            for dc in range(DC):
                nc.vector.tensor_copy(
                    out=x_fm[dc][:, 2+t0:2+t0+128],
                    in_=tp_ps[:, dc*128:(dc+1)*128])
        return mcols

    def alloc_rowbuf():
        x_fm = [ep.tile([128, c.S + 2], bf16, name=f"xfm_{dcc}", bufs=1)
                for dcc in range(DC)]
        y1_fm = [ep.tile([128, c.S + 2], bf16, name=f"y1fm_{dcc}", bufs=1)
                 for dcc in range(DC)]
        for dc in range(DC):
            nc.vector.tensor_copy(out=x_fm[dc][:, 0:2], in_=zpad[:, :])
            nc.vector.tensor_copy(out=y1_fm[dc][:, 0:2], in_=zpad[:, :])
        return x_fm, y1_fm

    # Hoist row-0 buffers + the first supertile's gather ahead of the
    # weight loads so the PE can start transposing at once.
    rowbuf = {0: alloc_rowbuf()}
    pre_mcols = emit_gather(0, 0, rowbuf[0][0])

    # weight loads split across both HWDGE queues (conv1 gates iter 0)
    w1t = wp.tile([128, K, DC, c.D], bf16, name="w1t")
    nc.sync.dma_start(out=w1t[:, 0:2, :, :], in_=c1w[:, 0:2, :, :])
    nc.scalar.dma_start(out=w1t[:, 2:3, :, :], in_=c1w[:, 2:3, :, :])
    w2t = wp.tile([128, K, DC, c.D], bf16, name="w2t")
    nc.scalar.dma_start(out=w2t[:, 0:2, :, :], in_=c2w[:, 0:2, :, :])
    nc.sync.dma_start(out=w2t[:, 2:3, :, :], in_=c2w[:, 2:3, :, :])
    gwt = gp.tile([128, DC, c.D], bf16, name="gwt")
    nc.sync.dma_start(out=gwt[:], in_=gwT[:])
    hwt = gp.tile([128, DC, c.D], bf16, name="hwt")
    nc.scalar.dma_start(out=hwt[:], in_=hw_w[:])

    # =================== GCN (emitted inside first supertile) ===========
    def emit_gcn():
        # X_tilde and A_hatT shard arrive pre-scaled / pre-transposed bf16.
        xtt = gp.tile([128, NJC, c.D], bf16, name="xtt")
        nc.sync.dma_start(out=xtt[:, 0:NJC//2, :], in_=X_t[:, 0:NJC//2, :])
        nc.scalar.dma_start(out=xtt[:, NJC//2:, :], in_=X_t[:, NJC//2:, :])
        att = gp.tile([128, NJC, NL], bf16, name="att")
        nc.scalar.dma_start(out=att[:, 0:NJC//2, :], in_=A_locT[:, 0:NJC//2, :])
        nc.sync.dma_start(out=att[:, NJC//2:, :], in_=A_locT[:, NJC//2:, :])

        # M1T[d, i] = sum_j Xt[j, d] * A_hatT[j, i]
        m1 = []
        for dc in range(DC):
            m1_ps = pe.tile([128, NL], f32, name="m1p", tag="c1", bufs=2)
            for jc in range(NJC):
                nc.tensor.matmul(out=m1_ps[:],
                                 lhsT=xtt[:, jc, dc*128:(dc+1)*128],
                                 rhs=att[:, jc, :], start=(jc == 0),
                                 stop=(jc == NJC - 1))
            m1s = gp.tile([128, NL], bf16, name=f"m1s_{dc}")
            nc.scalar.copy(out=m1s[:], in_=m1_ps[:])
            m1.append(m1s)

        # d_i broadcast [128, N_loc]
        dl_f = gp.tile([1, NL], f32, name="dl_f")
        nc.scalar.dma_start(out=dl_f[:], in_=d_loc[0:1, :])
        dl_r = gp.tile([1, NL], f32r, name="dl_r")
        nc.vector.tensor_copy(out=dl_r[:], in_=dl_f[:])
        db_ps = pe.tile([128, NL], f32, name="db_ps", tag="lg", bufs=2)
        nc.tensor.matmul(out=db_ps[:], lhsT=ones_row[:], rhs=dl_r[:],
                         start=True, stop=True)
        db = gp.tile([128, NL], f32, name="db")
        nc.vector.tensor_copy(out=db[:], in_=db_ps[:])

        # H_shard = relu(d_i * (gw @ M1T) + gcn_b)   [e, i] bf16
        HT = []
        for ec in range(DC):
            h2_ps = pe.tile([128, NL], f32, name=f"h2p_{ec}", tag="lg",
                            bufs=2)
            for dc in range(DC):
                nc.tensor.matmul(out=h2_ps[:],
                                 lhsT=gwt[:, dc, ec*128:(ec+1)*128],
                                 rhs=m1[dc][:], start=(dc == 0),
                                 stop=(dc == DC - 1))
            hd = gp.tile([128, NL], f32, name="hd", bufs=2)
            nc.vector.tensor_mul(hd[:], h2_ps[:], db[:])
            ht = gp.tile([128, NL], bf16, name=f"ht_{ec}")
            nc.scalar.activation(out=ht[:], in_=hd[:], func=AF.Relu,
                                 bias=gb_col[ec][:])
            HT.append(ht)

        # G_shard^T[d, i] = sum_e W[e, d] * H_shard[e, i]; scale by gamma[d]
        gg_in = dram.tile([c.D, NL], bf16)
        gg_all = dram.tile([c.n_cores, c.D, NL], bf16, addr_space="Shared")
        for dc in range(DC):
            g2_ps = pe.tile([128, NL], f32, name=f"g2p_{dc}", tag="lg",
                            bufs=2)
            for ec in range(DC):
                nc.tensor.matmul(out=g2_ps[:],
                                 lhsT=hwt[:, ec, dc*128:(dc+1)*128],
                                 rhs=HT[ec][:], start=(ec == 0),
                                 stop=(ec == DC - 1))
            gg_sb = gp.tile([128, NL], bf16, name="gg_sb", bufs=2)
            if has_c0:
                nc.vector.tensor_copy(out=gg_sb[:], in_=g2_ps[:])
            else:
                nc.vector.tensor_scalar_mul(gg_sb[:], g2_ps[:], g_col[dc][:])
            nc.gpsimd.dma_start(out=gg_in[dc*128:(dc+1)*128, :], in_=gg_sb[:])

        nc.gpsimd.collective_compute(
            "AllGather", mybir.AluOpType.bypass,
            replica_groups=[list(range(c.n_cores))],
            ins=[gg_in[:].opt()],
            outs=[gg_all[:].opt()])

        # read back c2-major so the first logits slice is ready earliest
        for c2 in range(c.n_cores):
            for dc in range(DC):
                eng = nc.scalar if (c2 * DC + dc) % 2 == 0 else nc.sync
                eng.dma_start(out=GgT[dc][:, c2, :],
                              in_=gg_all[c2, dc*128:(dc+1)*128, :])

    # c0 row broadcast tiles (only when ln_b != 0)
    c0_bc = []

    def emit_c0():
        lnb_r = []
        for dc in range(DC):
            lr = ep.tile([128, 1], bf16, name=f"lnb_r_{dc}")
            nc.vector.tensor_copy(out=lr[:], in_=lnb_col[dc][:])
            lnb_r.append(lr)
        for ns in range(NSL):
            c0_ps = pe.tile([1, SW], f32, name="c0_ps", tag="sbc", bufs=2)
            for dc in range(DC):
                nc.tensor.matmul(out=c0_ps[0:1, :], lhsT=lnb_r[dc][:],
                                 rhs=GgT[dc][:, 2*ns:2*ns+2, :],
                                 start=(dc == 0), stop=(dc == DC - 1))
            c0_row = ep.tile([1, SW], f32r, name="c0_row", bufs=2)
            nc.vector.tensor_copy(out=c0_row[:], in_=c0_ps[:])
            cb_ps = pe.tile([128, SW], f32, name="cb_ps", tag="sbc", bufs=2)
            nc.tensor.matmul(out=cb_ps[:], lhsT=ones_row[:],
                             rhs=c0_row[0:1, :], start=True, stop=True)
            cb = ep.tile([128, SW], f32, name=f"c0bc_{ns}")
            nc.vector.tensor_copy(out=cb[:], in_=cb_ps[:])
            c0_bc.append(cb)

    # =================== encode + logits =================
    iters = [(row, st) for row in range(c.B_loc) for st in range(NST)]
    zc_pend = [None]
    lg_pend = [None, None, None]

    for i, (row, st) in enumerate(iters):
        if st == 0 and row not in rowbuf:
            rowbuf[row] = alloc_rowbuf()
        x_fm, y1_fm = rowbuf[row]
        row0, s0 = row * c.S, st * ST

        # ---- gather + transposes (hoisted for the very first supertile) --
        if i == 0:
            mcols = pre_mcols
        else:
            mcols = emit_gather(row, st, x_fm)

        # ---- zc of ST t-1 (DVE), then logits of ST t-2 (PE) ----
        if zc_pend[0] is not None:
            zc_pend[0]()
            zc_pend[0] = None
        if lg_pend[0] is not None:
            lg_pend[0]()
        lg_pend[0] = lg_pend[1]
        lg_pend[1] = lg_pend[2]
        lg_pend[2] = None

        # ---- conv1 (fm out) + relu ----
        for dc in range(DC):
            c1_ps = pe.tile([128, ST], f32, name="c1_ps", tag="c1", bufs=2)
            first = True
            for k in range(K):
                for dci in range(DC):
                    nc.tensor.matmul(
                        out=c1_ps[:],
                        lhsT=w1t[:, k, dci, dc*128:(dc+1)*128],
                        rhs=x_fm[dci][:, s0+k:s0+k+ST],
                        start=first, stop=(k == K-1 and dci == DC-1))
                    first = False
            nc.scalar.activation(out=y1_fm[dc][:, 2+s0:2+s0+ST],
                                 in_=c1_ps[:], func=AF.Relu,
                                 bias=b1_col[dc][:])

        if i == 0:
            emit_gcn()

        # ---- conv2 (fm out) + bias + residual -> z ----
        z = []
        for dc in range(DC):
            c2_ps = pe.tile([128, ST], f32, name="c2_ps", tag="c2s", bufs=2)
            first = True
            for k in range(K):
                for dci in range(DC):
                    nc.tensor.matmul(
                        out=c2_ps[:],
                        lhsT=w2t[:, k, dci, dc*128:(dc+1)*128],
                        rhs=y1_fm[dci][:, s0+k:s0+k+ST],
                        start=first, stop=(k == K-1 and dci == DC-1))
                    first = False
            zt = ep.tile([128, ST], bf16, name=f"z_{dc}", bufs=2)
            nc.vector.scalar_tensor_tensor(
                out=zt[:], in0=c2_ps[:], scalar=b2_col[dc][:],
                in1=x_fm[dc][:, 2+s0:2+s0+ST],
                op0=OP.add, op1=OP.add)
            z.append(zt)

        # ---- LN stats via PE matvecs (sum, then sumsq) ----
        st_ps = pe.tile([1, ST], f32, name="st_ps", tag="sbc", bufs=2)
        for dc in range(DC):
            nc.tensor.matmul(out=st_ps[0:1, :], lhsT=ones_col[:],
                             rhs=z[dc][:], start=(dc == 0),
                             stop=(dc == DC-1))
        zsq = []
        for dc in range(DC):
            zq = ep.tile([128, ST], bf16, name="zsq", bufs=2)
            nc.scalar.square(zq[:], z[dc][:])
            zsq.append(zq)
        sq_ps = pe.tile([1, ST], f32, name="sq_ps", tag="sbc", bufs=2)
        for dc in range(DC):
            nc.tensor.matmul(out=sq_ps[0:1, :], lhsT=ones_col[:],
                             rhs=zsq[dc][:], start=(dc == 0),
                             stop=(dc == DC-1))

        # ---- LN: mu, var; per-token-tile rstd*mask columns ----
        mu = ep.tile([1, ST], f32r, name="mu", bufs=2)
        nc.scalar.mul(mu[:], st_ps[0:1, :], 1.0 / c.D)
        ms = ep.tile([1, ST], f32, name="ms", bufs=2)
        nc.scalar.mul(ms[:], sq_ps[0:1, :], 1.0 / c.D)
        musq = ep.tile([1, ST], f32, name="musq", bufs=2)
        nc.scalar.square(musq[:], mu[:].bitcast(f32))
        nc.vector.scalar_tensor_tensor(out=ms[:], in0=ms[:],
                                       scalar=LN_EPS, in1=musq[:],
                                       op0=OP.add, op1=OP.subtract)
        rsm = []
        for tt in range(NTT):
            v_ps = pe.tile([128, 1], f32, name="v_ps", tag="sbc", bufs=2)
            nc.tensor.transpose(out=v_ps[:],
                                in_=ms[0:1, tt*128:(tt+1)*128],
                                identity=ident[0:1, 0:1])
            vr = ep.tile([128, 1], f32, name="vr", bufs=4)
            nc.vector.reciprocal(vr[:], v_ps[:])
            rs = ep.tile([128, 1], f32, name="rs", bufs=4)
            nc.scalar.sqrt(rs[:], vr[:])
            rm = ep.tile([128, 1], f32, name="rm", bufs=16)
            nc.vector.tensor_mul(rm[:], rs[:], mcols[tt][:])
            rsm.append(rm)

        # mu broadcast for the zc subtraction (consumed next iteration)
        mu_bc = pe.tile([128, ST], f32, name="mu_bc", tag="sbc", bufs=2)
        nc.tensor.matmul(out=mu_bc[:], lhsT=ones_row[:], rhs=mu[:],
                         start=True, stop=True)

        zc = [ep.tile([128, ST], bf16, name=f"zc_{dc}", bufs=4)
              for dc in range(DC)]

        def emit_zc(z=z, zc=zc, mu_bc=mu_bc):
            for dc in range(DC):
                if has_c0:
                    t1 = ep.tile([128, ST], f32, name="zc_t1", bufs=2)
                    nc.vector.tensor_tensor(out=t1[:], in0=z[dc][:],
                                            in1=mu_bc[:], op=OP.subtract)
                    nc.vector.tensor_scalar_mul(zc[dc][:], t1[:],
                                                g_col[dc][:])
                else:
                    nc.vector.tensor_tensor(out=zc[dc][:], in0=z[dc][:],
                                            in1=mu_bc[:], op=OP.subtract)

        def emit_logits(zc=zc, rsm=rsm, row0=row0, s0=s0, mcols=mcols):
            for tt in range(NTT):
                lo = ep.tile([128, c.N], bf16, name="lo", bufs=3)
                for ns in range(NSL):
                    lg_ps = pe.tile([128, SW], f32, name="lg_ps", tag="lg",
                                    bufs=2)
                    for dc in range(DC):
                        nc.tensor.matmul(
                            out=lg_ps[:],
                            lhsT=zc[dc][:, tt*128:(tt+1)*128],
                            rhs=GgT[dc][:, 2*ns:2*ns+2, :],
                            start=(dc == 0), stop=(dc == DC-1))
                    if has_c0:
                        t2 = ep.tile([128, SW], f32, name="lg_t2", bufs=2)
                        nc.vector.tensor_scalar_mul(t2[:], lg_ps[:],
                                                    rsm[tt][:])
                        nc.vector.scalar_tensor_tensor(
                            out=lo[:, ns*SW:(ns+1)*SW], in0=c0_bc[ns][:],
                            scalar=mcols[tt][:], in1=t2[:],
                            op0=OP.mult, op1=OP.add)
                    elif ns % 2 == 0:
                        nc.scalar.mul(lo[:, ns*SW:(ns+1)*SW], lg_ps[:],
                                      rsm[tt][:])
                    else:
                        nc.vector.tensor_scalar_mul(lo[:, ns*SW:(ns+1)*SW],
                                                    lg_ps[:], rsm[tt][:])
                t0g = row0 + s0 + tt * 128
                nc.sync.dma_start(out=out[t0g:t0g+128, :], in_=lo[:])

        zc_pend[0] = emit_zc
        lg_pend[2] = emit_logits

        if i == 0 and has_c0:
            emit_c0()

    if zc_pend[0] is not None:
        zc_pend[0]()
    for fn in lg_pend:
        if fn is not None:
            fn()

    pe.release()
    ep.release()
    dram.release()
    gp.release()
    wp.release()
    ggp.release()
    const.release()


# ---------------------------------------------------------------------------
# host side
# ---------------------------------------------------------------------------

_CACHE = {}
_BF16 = mybir.dt.np(mybir.dt.bfloat16)


def _get_program(cfg, has_c0=False):
    key = (cfg.V, cfg.D, cfg.B, cfg.S, cfg.N, cfg.K, cfg.n_cores, has_c0)
    if key not in _CACHE:
        _CACHE[key] = build_program(cfg, has_c0=has_c0)
    return _CACHE[key]


class _Runner:
    """Direct PJRT execution (no donation) so repeated runs are cheap."""

    def __init__(self, nc, n_cores):
        import jax
        from jax.sharding import Mesh, PartitionSpec, NamedSharding
        from jax.experimental.shard_map import shard_map
        from concourse import bass2jax
        bass2jax.install_neuronx_cc_hook()
        self.jax = jax
        self.n_cores = n_cores
        part_name = nc.partition_id_tensor.name if nc.partition_id_tensor else None
        in_names, out_names, out_avals, zero_outs = [], [], [], []
        for alloc in nc.m.functions[0].allocations:
            if not isinstance(alloc, mybir.MemoryLocationSet):
                continue
            name = alloc.memorylocations[0].name
            if alloc.kind == "ExternalInput":
                if name != part_name:
                    in_names.append(name)
            elif alloc.kind == "ExternalOutput":
                out_names.append(name)
                shape = tuple(alloc.tensor_shape)
                dtype = mybir.dt.np(alloc.dtype)
                out_avals.append(jax.core.ShapedArray(shape, dtype))
                zero_outs.append(np.zeros(shape, dtype))
        self.in_names, self.out_names = in_names, out_names
        self.out_avals, self.zero_outs = out_avals, zero_outs
        self.n_params = len(in_names)
        all_in = list(in_names) + list(out_names)
        if part_name:
            all_in.append(part_name)
        out_avals_t, all_in_t, out_names_t = (tuple(out_avals), tuple(all_in),
                                              tuple(out_names))

        def _body(*args):
            operands = list(args)
            if part_name:
                operands.append(bass2jax.partition_id_tensor())
            return tuple(bass2jax._bass_exec_p.bind(
                *operands, out_avals=out_avals_t, in_names=all_in_t,
                out_names=out_names_t, lowering_input_output_aliases=(),
                sim_require_finite=True, sim_require_nnan=True, nc=nc))

        devices = jax.devices()[:n_cores]
        self.mesh = Mesh(np.asarray(devices), ("core",))
        n_io = self.n_params + len(out_names)
        self.sharded = jax.jit(
            shard_map(_body, mesh=self.mesh,
                      in_specs=(PartitionSpec("core"),) * n_io,
                      out_specs=(PartitionSpec("core"),) * len(out_names),
                      check_rep=False),
            keep_unused=True)
        self.shard = NamedSharding(self.mesh, PartitionSpec("core"))

    def set_inputs(self, in_maps):
        jax = self.jax
        per_core = [[np.asarray(m[n]) for n in self.in_names] for m in in_maps]
        concat = [np.concatenate([per_core[cc][i] for cc in range(self.n_cores)],
                                 axis=0) for i in range(self.n_params)]
        concat += [np.zeros((self.n_cores * z.shape[0], *z.shape[1:]), z.dtype)
                   for z in self.zero_outs]
        self.dev_in = [jax.device_put(a, self.shard) for a in concat]
        jax.block_until_ready(self.dev_in)

    def run(self):
        outs = self.sharded(*self.dev_in)
        self.jax.block_until_ready(outs)
        return outs

    def run_np(self):
        outs = self.run()
        return [
            {n: np.asarray(outs[i]).reshape(self.n_cores,
                                            *self.out_avals[i].shape)[cc]
             for i, n in enumerate(self.out_names)}
            for cc in range(self.n_cores)
        ]


_RUNNER = {}


def _pmajor(arr, npart=128):
    """[n*128, rest...] -> [128, n, rest...] partition-major relayout."""
    n = arr.shape[0] // npart
    return np.ascontiguousarray(
        arr.reshape(n, npart, *arr.shape[1:]).swapaxes(0, 1))


def make_in_maps(cfg, inputs):
    c = cfg
    x_in = np.asarray(inputs['x_in'])
    mask = np.asarray(inputs['mask_in']).astype(np.float32)
    A = np.asarray(inputs['A_sub']).astype(np.float32)
    embed = np.asarray(inputs['embed']).astype(np.float32)
    sub_nodes = np.asarray(inputs['sub_nodes']).astype(np.int64)
    d_full = 1.0 / np.sqrt(np.maximum(A.sum(axis=1) + 1.0, 1e-6))
    d_full = d_full.astype(np.float32)
    X_t = (embed[sub_nodes] * d_full[:, None]).astype(_BF16)
    gcn_wT = np.ascontiguousarray(np.asarray(inputs['gcn_w']).T)
    w1 = np.asarray(inputs['conv1_w']).astype(_BF16)   # [K, D, D]
    w2 = np.asarray(inputs['conv2_w']).astype(_BF16)
    # [K, D, D] -> [128, K, DC, D]
    def conv_pm(w):
        return np.ascontiguousarray(
            w.reshape(c.K, c.DC, 128, c.D).transpose(2, 0, 1, 3))
    shared = {
        'X_tp': _pmajor(X_t),
        'embed_bf': embed.astype(_BF16),
        'conv1_wp': conv_pm(w1),
        'conv1_b': np.asarray(inputs['conv1_b']).astype(np.float32),
        'conv2_wp': conv_pm(w2),
        'conv2_b': np.asarray(inputs['conv2_b']).astype(np.float32),
        'ln_g': np.asarray(inputs['ln_g']).astype(np.float32),
        'ln_b': np.asarray(inputs['ln_b']).astype(np.float32),
        'gcn_wTp': _pmajor(gcn_wT.astype(_BF16)),
        'gcn_b': np.asarray(inputs['gcn_b']).astype(np.float32),
        'hproj_wp': _pmajor(np.asarray(inputs['hproj_w']).astype(_BF16)),
    }
    in_maps = []
    for cc in range(c.n_cores):
        rows = slice(cc * c.B_loc, (cc + 1) * c.B_loc)
        nrows = slice(cc * c.N_loc, (cc + 1) * c.N_loc)
        A_loc = A[nrows].copy()
        A_loc[np.arange(c.N_loc), np.arange(cc * c.N_loc,
                                            (cc + 1) * c.N_loc)] += 1.0
        m = dict(shared)
        m['x_in_loc'] = np.ascontiguousarray(
            x_in[rows].reshape(-1)).astype(np.int32)
        m['mask_loc'] = np.ascontiguousarray(mask[rows].reshape(-1))
        m['A_locTp'] = _pmajor(
            np.ascontiguousarray(A_loc.T).astype(_BF16))
        m['d_loc'] = d_full[nrows].reshape(1, -1).copy()
        in_maps.append(m)
    return in_maps


def kernel(**inputs):
    cfg = Cfg()
    has_c0 = bool(np.any(np.asarray(inputs['ln_b']) != 0))
    nc = _get_program(cfg, has_c0)
    key = id(nc)
    if key not in _RUNNER:
        _RUNNER[key] = _Runner(nc, cfg.n_cores)
    r = _RUNNER[key]
    r.set_inputs(make_in_maps(cfg, inputs))
    res = r.run_np()
    out = np.concatenate(
        [res[cc]['logits_loc'].astype(np.float32).reshape(cfg.B_loc, cfg.S,
                                                          cfg.N)
         for cc in range(cfg.n_cores)], axis=0)
    return out
